# revision 10
# baseline (speedup 1.0000x reference)
"""Trainium2 Bass kernel for the Alignment (ESIM-style) module.

Math (per batch):
    a_out = relu(a_in @ W1.T + b1); b_out = relu(b_in @ W2.T + b2)
    S = (a_out @ b_out.T) * temperature              [L, L]
    E = exp(S + ln(am_i)) * bm_j   (fully masked exp; no max-subtraction:
        scores are O(+-3) so fp32 exp is safe; exp(-1e9)==0 matches *0)
    a_feature[j, d] = sum_i E[i,j]*a_in[i,d] / sum_i E[i,j]   (bm_j cancels)
    b_feature[i, d] = sum_j E[i,j]*b_in[j,d] / sum_j E[i,j]   (am_i cancels)
    Matches the reference -1e9-mask softmax exactly for non-degenerate masks.

Sharding: pure data-parallel over batch. B=16 over 8 cores -> 2 batches/core.
All matmuls bf16 (fp32 PSUM accumulate); all transposes on the TensorEngine,
with transpose outputs packed 8-to-a-PSUM-bank and copied out in one op.
Softmax denominators come from DVE reduces (no extra matmuls).
"""

import numpy as np

_NC_CACHE = {}


def _build(B_local, L, D, H, reps=1):
    import concourse.mybir as mybir
    from concourse import bacc
    from concourse.tile import TileContext
    from concourse.masks import make_identity

    F32 = mybir.dt.float32
    BF16 = mybir.dt.bfloat16
    AF = mybir.ActivationFunctionType
    ALU = mybir.AluOpType

    assert L % 512 == 0 and D % 512 == 0 and H % 128 == 0
    DT = D // 128   # d-tiles (proj contraction)
    HT = H // 128   # h-tiles (scores contraction)
    LI = L // 128   # 128-row blocks of the sequence
    NC = L // 512   # 512-col chunks of the sequence
    DC = D // 512   # 512-col chunks of the feature dim
    CG = 1          # chunks per load group
    TG = min(8, DT)  # transposes packed per PSUM bank (input path)
    EG = min(8, LI)  # transposes packed per PSUM bank (E path)
    assert LI % CG == 0 and DT % TG == 0 and LI % EG == 0

    nc = bacc.Bacc()

    a_in = nc.declare_dram_parameter("a_inputs", [B_local, L, D], F32, isOutput=False)
    a_m = nc.declare_dram_parameter("a_mask", [B_local, L, 1], F32, isOutput=False)
    b_in = nc.declare_dram_parameter("b_inputs", [B_local, L, D], F32, isOutput=False)
    b_m = nc.declare_dram_parameter("b_mask", [B_local, L, 1], F32, isOutput=False)
    W1 = nc.declare_dram_parameter("W1", [H, D], F32, isOutput=False)
    b1 = nc.declare_dram_parameter("b1", [H], F32, isOutput=False)
    W2 = nc.declare_dram_parameter("W2", [H, D], F32, isOutput=False)
    b2 = nc.declare_dram_parameter("b2", [H], F32, isOutput=False)
    temp = nc.declare_dram_parameter("temperature", [1], F32, isOutput=False)
    out = nc.declare_dram_parameter("out", [B_local, 2, L, D], F32, isOutput=True)

    with TileContext(nc) as tc:
        with (
            tc.tile_pool(name="const", bufs=1) as cp,
            tc.tile_pool(name="unit", bufs=4) as up,
            tc.tile_pool(name="chunk", bufs=2) as chp,
            tc.tile_pool(name="eblk", bufs=2) as ebp,
            tc.tile_pool(name="feat", bufs=2) as fp,
            tc.tile_pool(name="bmbc", bufs=1) as bmp,
            tc.tile_pool(name="small", bufs=2) as sp,
            tc.tile_pool(name="psmm", bufs=3, space="PSUM") as psmm,
            tc.tile_pool(name="psacc", bufs=3, space="PSUM") as psacc,
            tc.tile_pool(name="pstp", bufs=2, space="PSUM") as pstp,
        ):
            # ---- constants ----
            ident = cp.tile([128, 128], BF16, tag="ident")
            make_identity(nc, ident)

            t_sb = cp.tile([1, 1], F32, tag="t_sb")
            nc.sync.dma_start(t_sb[:], temp[None, :])
            t_bc = cp.tile([128, 1], F32, tag="t_bc")
            nc.gpsimd.partition_broadcast(t_bc[:], t_sb[:])

            b1_sb = cp.tile([128, HT], F32, tag="b1_sb")
            nc.sync.dma_start(b1_sb[:], b1.rearrange("(t p) -> p t", p=128))
            b1_eff = cp.tile([128, HT], F32, tag="b1_eff")
            nc.vector.tensor_scalar_mul(b1_eff[:], b1_sb[:], t_bc[:])
            b2_sb = cp.tile([128, HT], F32, tag="b2_sb")
            nc.sync.dma_start(b2_sb[:], b2.rearrange("(t p) -> p t", p=128))

            # ---- weights: W [H, D] f32 -> WT [d%128, DT, H] bf16 (PE transpose) ----
            WTs = []
            for wi, W in enumerate((W1, W2)):
                WT = cp.tile([128, DT, H], BF16, tag=f"W{wi}T")
                for ht in range(HT):
                    wch = chp.tile([128, 1, D], F32, tag="chunk")
                    nc.sync.dma_start(wch[:, 0, :], W[ht * 128:(ht + 1) * 128, :])
                    wchb = sp.tile([128, D], BF16, tag="chunkb")
                    nc.vector.tensor_copy(wchb[:], wch[:, 0, :])
                    for kg in range(DT // TG):
                        ptp = pstp.tile([128, TG, 128], BF16, tag="tp")
                        for k in range(TG):
                            kt = kg * TG + k
                            nc.tensor.transpose(
                                ptp[:, k, :],
                                wchb[:, kt * 128:(kt + 1) * 128],
                                ident[:],
                            )
                        nc.vector.tensor_copy(
                            WT[:, kg * TG:(kg + 1) * TG, ht * 128:(ht + 1) * 128],
                            ptp[:],
                        )
                WTs.append(WT)
            W1T, W2T = WTs

            for rep, b in [(r, bb) for r in range(reps) for bb in range(B_local)]:
                # ---- masks ----
                am_all = sp.tile([128, LI], F32, tag="mrow")
                nc.sync.dma_start(
                    am_all[:], a_m.rearrange("b (c p) one -> b p (c one)", p=128)[b]
                )
                lnam = sp.tile([128, LI], F32, tag="lnam")
                nc.scalar.activation(lnam[:], am_all[:], AF.Ln)
                bm_row = sp.tile([1, L], F32, tag="bmrow", bufs=1)
                nc.sync.dma_start(bm_row[:], b_m.rearrange("b l one -> b (one l)")[b][None, :])
                bm_row_bf = sp.tile([1, L], BF16, tag="bmrowb", bufs=1)
                nc.vector.tensor_copy(bm_row_bf[:], bm_row[:])
                bm_bc = bmp.tile([128, L], BF16, tag="bmbc", name=f"bmbc{b}")
                nc.gpsimd.partition_broadcast(bm_bc[:], bm_row_bf[:])

                # ---- phase A: load + convert + transpose both sides ----
                nat = [None, None]
                inT = [None, None]
                for side in range(2):  # 0 = a, 1 = b
                    src = a_in if side == 0 else b_in
                    nat[side] = up.tile(
                        [128, LI, D], BF16, tag="unit", name=f"nat{side}_{b}"
                    )
                    inT[side] = up.tile(
                        [128, DT, L], BF16, tag="unit", name=f"inT{side}_{b}"
                    )
                    for g in range(LI // CG):
                        ch = chp.tile([128, CG, D], F32, tag="chunk")
                        nc.sync.dma_start(
                            ch[:],
                            src.rearrange("b (g c p) d -> b g p c d", p=128, c=CG)[b, g],
                        )
                        nc.vector.tensor_copy(
                            nat[side][:, g * CG:(g + 1) * CG, :], ch[:]
                        )
                        for c in range(g * CG, (g + 1) * CG):
                            for kg in range(DT // TG):
                                ptp = pstp.tile([128, TG, 128], BF16, tag="tp")
                                for k in range(TG):
                                    kt = kg * TG + k
                                    nc.tensor.transpose(
                                        ptp[:, k, :],
                                        nat[side][:, c, kt * 128:(kt + 1) * 128],
                                        ident[:],
                                    )
                                nc.vector.tensor_copy(
                                    inT[side][:, kg * TG:(kg + 1) * TG,
                                              c * 128:(c + 1) * 128],
                                    ptp[:],
                                )

                # ---- phase B: projections -> outT [h, i] ----
                outT = up.tile([128, 2, HT, L], BF16, tag="unit", name=f"outT{b}")
                for side in range(2):
                    WT = W1T if side == 0 else W2T
                    for ht in range(HT):
                        for icn in range(NC):
                            pm = psmm.tile([128, 512], F32, tag="mm")
                            for kt in range(DT):
                                nc.tensor.matmul(
                                    pm[:],
                                    WT[:, kt, ht * 128:(ht + 1) * 128],
                                    inT[side][:, kt, icn * 512:(icn + 1) * 512],
                                    start=(kt == 0),
                                    stop=(kt == DT - 1),
                                )
                            if side == 0:
                                # fold temperature: relu(t*x + t*b1) = t*relu(x+b1)
                                nc.scalar.activation(
                                    outT[:, 0, ht, icn * 512:(icn + 1) * 512], pm[:],
                                    AF.Relu, bias=b1_eff[:, ht:ht + 1], scale=t_bc[:],
                                )
                            else:
                                nc.scalar.activation(
                                    outT[:, 1, ht, icn * 512:(icn + 1) * 512], pm[:],
                                    AF.Relu, bias=b2_sb[:, ht:ht + 1], scale=1.0,
                                )

                # ---- pass 1 (i-outer): E row blocks (masked), b_feature ----
                E_half = [
                    up.tile([128, LI // 2, L], BF16, tag="unit", name=f"E{h}_{b}")
                    for h in range(2)
                ]
                c_acc = sp.tile([128, LI], F32, tag="cacc")
                for iblk in range(LI):
                    Et = E_half[0] if iblk < LI // 2 else E_half[1]
                    ei = iblk % (LI // 2)
                    for jc in range(NC):
                        ps = psmm.tile([128, 512], F32, tag="mm")
                        for kt in range(HT):
                            nc.tensor.matmul(
                                ps[:],
                                outT[:, 0, kt, iblk * 128:(iblk + 1) * 128],
                                outT[:, 1, kt, jc * 512:(jc + 1) * 512],
                                start=(kt == 0),
                                stop=(kt == HT - 1),
                            )
                        nc.scalar.activation(
                            Et[:, ei, jc * 512:(jc + 1) * 512], ps[:], AF.Exp,
                            bias=lnam[:, iblk:iblk + 1],
                        )
                    # row-mask by bm (in-place), then row sums r
                    # (tensor_tensor_reduce would fuse these but crashes on HW)
                    r_blk = sp.tile([128, 1], F32, tag="rblk", bufs=4)
                    nc.vector.tensor_mul(Et[:, ei, :], Et[:, ei, :], bm_bc[:])
                    nc.vector.reduce_sum(
                        r_blk[:], Et[:, ei, :], axis=mybir.AxisListType.X, 
                    )
                    # transpose the (masked) row block
                    EbT = ebp.tile([128, LI, 128], BF16, tag="eblk")
                    for jg in range(LI // EG):
                        ptp = pstp.tile([128, EG, 128], BF16, tag="tp", name="ptpe")
                        for k in range(EG):
                            jt = jg * EG + k
                            nc.tensor.transpose(
                                ptp[:, k, :], Et[:, ei, jt * 128:(jt + 1) * 128],
                                ident[:],
                            )
                        nc.vector.tensor_copy(EbT[:, jg * EG:(jg + 1) * EG, :], ptp[:])
                    # column-sum contribution of this row block: c += sum_i EbT
                    c_part = sp.tile([128, LI, 1], F32, tag="cpart")
                    nc.vector.tensor_reduce(
                        c_part[:], EbT[:], axis=mybir.AxisListType.X, op=ALU.add,
                    )
                    if iblk == 0:
                        nc.vector.tensor_copy(c_acc[:], c_part[:, :, 0])
                    else:
                        nc.vector.tensor_add(c_acc[:], c_acc[:], c_part[:, :, 0])
                    # b_feature rows for this block
                    pn = [
                        psacc.tile([128, 512], F32, tag="acc", name=f"pn{dd}")
                        for dd in range(DC)
                    ]
                    for jt in range(LI):
                        for dcn in range(DC):
                            nc.tensor.matmul(
                                pn[dcn][:],
                                EbT[:, jt, :],
                                nat[1][:, jt, dcn * 512:(dcn + 1) * 512],
                                start=(jt == 0),
                                stop=(jt == LI - 1),
                            )
                    rv = sp.tile([128, 1], F32, tag="rinv", bufs=4)
                    nc.vector.reciprocal(rv[:], r_blk[:])
                    fo = fp.tile([128, D], F32, tag="feat")
                    for dcn in range(DC):
                        nc.scalar.activation(
                            fo[:, dcn * 512:(dcn + 1) * 512], pn[dcn][:],
                            AF.Copy, scale=rv[:],
                        )
                    nc.sync.dma_start(out[b, 1, iblk * 128:(iblk + 1) * 128, :], fo[:])

                # ---- pass 2 (j-outer): a_feature (rhs = unmasked a_in, reload) ----
                nat2 = up.tile([128, LI, D], BF16, tag="unit", name=f"nat2_{b}")
                for g in range(LI // CG):
                    ch = chp.tile([128, CG, D], F32, tag="chunk")
                    nc.sync.dma_start(
                        ch[:],
                        a_in.rearrange("b (g c p) d -> b g p c d", p=128, c=CG)[b, g],
                    )
                    nc.vector.tensor_copy(nat2[:, g * CG:(g + 1) * CG, :], ch[:])
                for jblk in range(LI):
                    pp = psacc if jblk % 2 == 0 else psmm
                    tg = "acc" if jblk % 2 == 0 else "mm"
                    pn = [
                        pp.tile([128, 512], F32, tag=tg, name=f"pa{dd}")
                        for dd in range(DC)
                    ]
                    for it in range(LI):
                        Et = E_half[0] if it < LI // 2 else E_half[1]
                        ei = it % (LI // 2)
                        for dcn in range(DC):
                            nc.tensor.matmul(
                                pn[dcn][:],
                                Et[:, ei, jblk * 128:(jblk + 1) * 128],
                                nat2[:, it, dcn * 512:(dcn + 1) * 512],
                                start=(it == 0),
                                stop=(it == LI - 1),
                            )
                    cv = sp.tile([128, 1], F32, tag="rinv", bufs=4)
                    nc.vector.reciprocal(cv[:], c_acc[:, jblk:jblk + 1])
                    fo = fp.tile([128, D], F32, tag="feat")
                    for dcn in range(DC):
                        nc.scalar.activation(
                            fo[:, dcn * 512:(dcn + 1) * 512], pn[dcn][:],
                            AF.Copy, scale=cv[:],
                        )
                    nc.sync.dma_start(out[b, 0, jblk * 128:(jblk + 1) * 128, :], fo[:])

    nc.compile()
    return nc


def get_nc(B_local=2, L=2048, D=1024, H=512, reps=1):
    key = (B_local, L, D, H, reps)
    if key not in _NC_CACHE:
        _NC_CACHE[key] = _build(*key[:4], reps=key[4])
    return _NC_CACHE[key]


def kernel(a_inputs, a_mask, b_inputs, b_mask, W1, b1, W2, b2, temperature):
    from concourse.bass_utils import run_bass_kernel_spmd

    n_cores = 8
    B, L, D = a_inputs.shape
    H = W1.shape[0]
    assert B % n_cores == 0
    B_local = B // n_cores
    nc = get_nc(B_local, L, D, H)

    f32 = lambda x: np.ascontiguousarray(np.asarray(x, dtype=np.float32))
    w_shared = {
        "W1": f32(W1), "b1": f32(b1), "W2": f32(W2), "b2": f32(b2),
        "temperature": f32(temperature).reshape(1),
    }
    in_maps = []
    for c in range(n_cores):
        sl = slice(c * B_local, (c + 1) * B_local)
        in_maps.append({
            "a_inputs": f32(a_inputs[sl]),
            "a_mask": f32(a_mask[sl]),
            "b_inputs": f32(b_inputs[sl]),
            "b_mask": f32(b_mask[sl]),
            **w_shared,
        })

    res = run_bass_kernel_spmd(nc, in_maps, core_ids=list(range(n_cores)))
    full = np.concatenate([res.results[c]["out"] for c in range(n_cores)], axis=0)
    a_feature = np.ascontiguousarray(full[:, 0])
    b_feature = np.ascontiguousarray(full[:, 1])
    return (a_feature, b_feature)


# revision 11
# speedup vs baseline: 189.2676x; 189.2676x over previous
"""Trainium2 Bass kernel for the Alignment (ESIM-style) module.

Math (per batch):
    a_out = relu(a_in @ W1.T + b1); b_out = relu(b_in @ W2.T + b2)
    S = (a_out @ b_out.T) * temperature              [L, L]
    E = exp(S + ln(am_i)) * bm_j   (fully masked exp; no max-subtraction:
        scores are O(+-3) so fp32 exp is safe; exp(-1e9)==0 matches *0)
    a_feature[j, d] = sum_i E[i,j]*a_in[i,d] / sum_i E[i,j]   (bm_j cancels)
    b_feature[i, d] = sum_j E[i,j]*b_in[j,d] / sum_j E[i,j]   (am_i cancels)
    Matches the reference -1e9-mask softmax exactly for non-degenerate masks.

Sharding: pure data-parallel over batch. B=16 over 8 cores -> 2 batches/core.
All matmuls bf16 (fp32 PSUM accumulate); all transposes on the TensorEngine,
with transpose outputs packed 8-to-a-PSUM-bank and copied out in one op.
Softmax denominators come from DVE reduces (no extra matmuls).
"""

import numpy as np

_NC_CACHE = {}


def _build(B_local, L, D, H, reps=1, phases='all'):
    import concourse.mybir as mybir
    from concourse import bacc
    from concourse.tile import TileContext
    from concourse.masks import make_identity

    F32 = mybir.dt.float32
    BF16 = mybir.dt.bfloat16
    AF = mybir.ActivationFunctionType
    ALU = mybir.AluOpType

    assert L % 512 == 0 and D % 512 == 0 and H % 128 == 0
    DT = D // 128   # d-tiles (proj contraction)
    HT = H // 128   # h-tiles (scores contraction)
    LI = L // 128   # 128-row blocks of the sequence
    NC = L // 512   # 512-col chunks of the sequence
    DC = D // 512   # 512-col chunks of the feature dim
    CG = 1          # chunks per load group
    TG = min(8, DT)  # transposes packed per PSUM bank (input path)
    EG = min(8, LI)  # transposes packed per PSUM bank (E path)
    assert LI % CG == 0 and DT % TG == 0 and LI % EG == 0

    nc = bacc.Bacc()

    a_in = nc.declare_dram_parameter("a_inputs", [B_local, L, D], F32, isOutput=False)
    a_m = nc.declare_dram_parameter("a_mask", [B_local, L, 1], F32, isOutput=False)
    b_in = nc.declare_dram_parameter("b_inputs", [B_local, L, D], F32, isOutput=False)
    b_m = nc.declare_dram_parameter("b_mask", [B_local, L, 1], F32, isOutput=False)
    W1 = nc.declare_dram_parameter("W1", [H, D], F32, isOutput=False)
    b1 = nc.declare_dram_parameter("b1", [H], F32, isOutput=False)
    W2 = nc.declare_dram_parameter("W2", [H, D], F32, isOutput=False)
    b2 = nc.declare_dram_parameter("b2", [H], F32, isOutput=False)
    temp = nc.declare_dram_parameter("temperature", [1], F32, isOutput=False)
    out = nc.declare_dram_parameter("out", [B_local, 2, L, D], F32, isOutput=True)

    with TileContext(nc) as tc:
        with (
            tc.tile_pool(name="const", bufs=1) as cp,
            tc.tile_pool(name="unit", bufs=4) as up,
            tc.tile_pool(name="chunk", bufs=2) as chp,
            tc.tile_pool(name="eblk", bufs=2) as ebp,
            tc.tile_pool(name="feat", bufs=2) as fp,
            tc.tile_pool(name="bmbc", bufs=1) as bmp,
            tc.tile_pool(name="small", bufs=2) as sp,
            tc.tile_pool(name="psmm", bufs=3, space="PSUM") as psmm,
            tc.tile_pool(name="psacc", bufs=3, space="PSUM") as psacc,
            tc.tile_pool(name="pstp", bufs=2, space="PSUM") as pstp,
        ):
            # ---- constants ----
            ident = cp.tile([128, 128], BF16, tag="ident")
            make_identity(nc, ident)

            t_sb = cp.tile([1, 1], F32, tag="t_sb")
            nc.sync.dma_start(t_sb[:], temp[None, :])
            t_bc = cp.tile([128, 1], F32, tag="t_bc")
            nc.gpsimd.partition_broadcast(t_bc[:], t_sb[:])

            b1_sb = cp.tile([128, HT], F32, tag="b1_sb")
            nc.sync.dma_start(b1_sb[:], b1.rearrange("(t p) -> p t", p=128))
            b1_eff = cp.tile([128, HT], F32, tag="b1_eff")
            nc.vector.tensor_scalar_mul(b1_eff[:], b1_sb[:], t_bc[:])
            b2_sb = cp.tile([128, HT], F32, tag="b2_sb")
            nc.sync.dma_start(b2_sb[:], b2.rearrange("(t p) -> p t", p=128))

            # ---- weights: W [H, D] f32 -> WT [d%128, DT, H] bf16 (PE transpose) ----
            WTs = []
            for wi, W in enumerate((W1, W2)):
                WT = cp.tile([128, DT, H], BF16, tag=f"W{wi}T")
                for ht in range(HT):
                    wch = chp.tile([128, 1, D], F32, tag="chunk")
                    nc.sync.dma_start(wch[:, 0, :], W[ht * 128:(ht + 1) * 128, :])
                    wchb = sp.tile([128, D], BF16, tag="chunkb")
                    nc.vector.tensor_copy(wchb[:], wch[:, 0, :])
                    for kg in range(DT // TG):
                        ptp = pstp.tile([128, TG, 128], BF16, tag="tp")
                        for k in range(TG):
                            kt = kg * TG + k
                            nc.tensor.transpose(
                                ptp[:, k, :],
                                wchb[:, kt * 128:(kt + 1) * 128],
                                ident[:],
                            )
                        nc.vector.tensor_copy(
                            WT[:, kg * TG:(kg + 1) * TG, ht * 128:(ht + 1) * 128],
                            ptp[:],
                        )
                WTs.append(WT)
            W1T, W2T = WTs

            for rep, b in [(r, bb) for r in range(reps) for bb in range(B_local)]:
                # ---- masks ----
                am_all = sp.tile([128, LI], F32, tag="mrow")
                nc.sync.dma_start(
                    am_all[:], a_m.rearrange("b (c p) one -> b p (c one)", p=128)[b]
                )
                lnam = sp.tile([128, LI], F32, tag="lnam")
                nc.scalar.activation(lnam[:], am_all[:], AF.Ln)
                bm_row = sp.tile([1, L], F32, tag="bmrow", bufs=1)
                nc.sync.dma_start(bm_row[:], b_m.rearrange("b l one -> b (one l)")[b][None, :])
                bm_row_bf = sp.tile([1, L], BF16, tag="bmrowb", bufs=1)
                nc.vector.tensor_copy(bm_row_bf[:], bm_row[:])
                bm_bc = bmp.tile([128, L], BF16, tag="bmbc", name=f"bmbc{b}")
                nc.gpsimd.partition_broadcast(bm_bc[:], bm_row_bf[:])

                # ---- phase A: load + convert + transpose both sides ----
                nat = [None, None]
                inT = [None, None]
                for side in range(2):  # 0 = a, 1 = b
                    src = a_in if side == 0 else b_in
                    nat[side] = up.tile(
                        [128, LI, D], BF16, tag="unit", name=f"nat{side}_{b}"
                    )
                    inT[side] = up.tile(
                        [128, DT, L], BF16, tag="unit", name=f"inT{side}_{b}"
                    )
                    for g in range(LI // CG):
                        ch = chp.tile([128, CG, D], F32, tag="chunk")
                        nc.sync.dma_start(
                            ch[:],
                            src.rearrange("b (g c p) d -> b g p c d", p=128, c=CG)[b, g],
                        )
                        nc.vector.tensor_copy(
                            nat[side][:, g * CG:(g + 1) * CG, :], ch[:]
                        )
                        for c in range(g * CG, (g + 1) * CG):
                            for kg in range(DT // TG):
                                ptp = pstp.tile([128, TG, 128], BF16, tag="tp")
                                for k in range(TG):
                                    kt = kg * TG + k
                                    nc.tensor.transpose(
                                        ptp[:, k, :],
                                        nat[side][:, c, kt * 128:(kt + 1) * 128],
                                        ident[:],
                                    )
                                nc.vector.tensor_copy(
                                    inT[side][:, kg * TG:(kg + 1) * TG,
                                              c * 128:(c + 1) * 128],
                                    ptp[:],
                                )

                # ---- phase B: projections -> outT [h, i] ----
                outT = up.tile([128, 2, HT, L], BF16, tag="unit", name=f"outT{b}")
                for side in range(2):
                    WT = W1T if side == 0 else W2T
                    for ht in range(HT):
                        for icn in range(NC):
                            pm = psmm.tile([128, 512], F32, tag="mm")
                            for kt in range(DT):
                                nc.tensor.matmul(
                                    pm[:],
                                    WT[:, kt, ht * 128:(ht + 1) * 128],
                                    inT[side][:, kt, icn * 512:(icn + 1) * 512],
                                    start=(kt == 0),
                                    stop=(kt == DT - 1),
                                )
                            if side == 0:
                                # fold temperature: relu(t*x + t*b1) = t*relu(x+b1)
                                nc.scalar.activation(
                                    outT[:, 0, ht, icn * 512:(icn + 1) * 512], pm[:],
                                    AF.Relu, bias=b1_eff[:, ht:ht + 1], scale=t_bc[:],
                                )
                            else:
                                nc.scalar.activation(
                                    outT[:, 1, ht, icn * 512:(icn + 1) * 512], pm[:],
                                    AF.Relu, bias=b2_sb[:, ht:ht + 1], scale=1.0,
                                )

                if phases == 'ab':
                    # dump outT so the phase isn't dead-code eliminated
                    foq = fp.tile([128, D], F32, tag="feat")
                    nc.vector.tensor_copy(foq[:, 0:L // 2 if False else 512], outT[:, 0, 0, 0:512])
                    nc.sync.dma_start(out[b, 0, 0:128, 0:512], foq[:, 0:512])
                    continue
                # ---- pass 1 (i-outer): E row blocks (masked), b_feature ----
                E_half = [
                    up.tile([128, LI // 2, L], BF16, tag="unit", name=f"E{h}_{b}")
                    for h in range(2)
                ]
                c_acc = sp.tile([128, LI], F32, tag="cacc")
                for iblk in range(LI):
                    Et = E_half[0] if iblk < LI // 2 else E_half[1]
                    ei = iblk % (LI // 2)
                    for jc in range(NC):
                        ps = psmm.tile([128, 512], F32, tag="mm")
                        for kt in range(HT):
                            nc.tensor.matmul(
                                ps[:],
                                outT[:, 0, kt, iblk * 128:(iblk + 1) * 128],
                                outT[:, 1, kt, jc * 512:(jc + 1) * 512],
                                start=(kt == 0),
                                stop=(kt == HT - 1),
                            )
                        nc.scalar.activation(
                            Et[:, ei, jc * 512:(jc + 1) * 512], ps[:], AF.Exp,
                            bias=lnam[:, iblk:iblk + 1],
                        )
                    # row-mask by bm (in-place), then row sums r
                    # (tensor_tensor_reduce would fuse these but crashes on HW)
                    r_blk = sp.tile([128, 1], F32, tag="rblk", bufs=4)
                    nc.vector.tensor_mul(Et[:, ei, :], Et[:, ei, :], bm_bc[:])
                    nc.vector.reduce_sum(
                        r_blk[:], Et[:, ei, :], axis=mybir.AxisListType.X, 
                    )
                    # transpose the (masked) row block
                    EbT = ebp.tile([128, LI, 128], BF16, tag="eblk")
                    for jg in range(LI // EG):
                        ptp = pstp.tile([128, EG, 128], BF16, tag="tp", name="ptpe")
                        for k in range(EG):
                            jt = jg * EG + k
                            nc.tensor.transpose(
                                ptp[:, k, :], Et[:, ei, jt * 128:(jt + 1) * 128],
                                ident[:],
                            )
                        nc.vector.tensor_copy(EbT[:, jg * EG:(jg + 1) * EG, :], ptp[:])
                    # column-sum contribution of this row block: c += sum_i EbT
                    c_part = sp.tile([128, LI, 1], F32, tag="cpart")
                    nc.vector.tensor_reduce(
                        c_part[:], EbT[:], axis=mybir.AxisListType.X, op=ALU.add,
                    )
                    if iblk == 0:
                        nc.vector.tensor_copy(c_acc[:], c_part[:, :, 0])
                    else:
                        nc.vector.tensor_add(c_acc[:], c_acc[:], c_part[:, :, 0])
                    # b_feature rows for this block
                    pn = [
                        psacc.tile([128, 512], F32, tag="acc", name=f"pn{dd}")
                        for dd in range(DC)
                    ]
                    for jt in range(LI):
                        for dcn in range(DC):
                            nc.tensor.matmul(
                                pn[dcn][:],
                                EbT[:, jt, :],
                                nat[1][:, jt, dcn * 512:(dcn + 1) * 512],
                                start=(jt == 0),
                                stop=(jt == LI - 1),
                            )
                    rv = sp.tile([128, 1], F32, tag="rinv", bufs=4)
                    nc.vector.reciprocal(rv[:], r_blk[:])
                    fo = fp.tile([128, D], F32, tag="feat")
                    for dcn in range(DC):
                        nc.scalar.activation(
                            fo[:, dcn * 512:(dcn + 1) * 512], pn[dcn][:],
                            AF.Copy, scale=rv[:],
                        )
                    nc.sync.dma_start(out[b, 1, iblk * 128:(iblk + 1) * 128, :], fo[:])

                if phases == 'abp1':
                    continue
                # ---- pass 2 (j-outer): a_feature (rhs = unmasked a_in, reload) ----
                nat2 = up.tile([128, LI, D], BF16, tag="unit", name=f"nat2_{b}")
                for g in range(LI // CG):
                    ch = chp.tile([128, CG, D], F32, tag="chunk")
                    nc.sync.dma_start(
                        ch[:],
                        a_in.rearrange("b (g c p) d -> b g p c d", p=128, c=CG)[b, g],
                    )
                    nc.vector.tensor_copy(nat2[:, g * CG:(g + 1) * CG, :], ch[:])
                for jblk in range(LI):
                    pp = psacc if jblk % 2 == 0 else psmm
                    tg = "acc" if jblk % 2 == 0 else "mm"
                    pn = [
                        pp.tile([128, 512], F32, tag=tg, name=f"pa{dd}")
                        for dd in range(DC)
                    ]
                    for it in range(LI):
                        Et = E_half[0] if it < LI // 2 else E_half[1]
                        ei = it % (LI // 2)
                        for dcn in range(DC):
                            nc.tensor.matmul(
                                pn[dcn][:],
                                Et[:, ei, jblk * 128:(jblk + 1) * 128],
                                nat2[:, it, dcn * 512:(dcn + 1) * 512],
                                start=(it == 0),
                                stop=(it == LI - 1),
                            )
                    cv = sp.tile([128, 1], F32, tag="rinv", bufs=4)
                    nc.vector.reciprocal(cv[:], c_acc[:, jblk:jblk + 1])
                    fo = fp.tile([128, D], F32, tag="feat")
                    for dcn in range(DC):
                        nc.scalar.activation(
                            fo[:, dcn * 512:(dcn + 1) * 512], pn[dcn][:],
                            AF.Copy, scale=cv[:],
                        )
                    nc.sync.dma_start(out[b, 0, jblk * 128:(jblk + 1) * 128, :], fo[:])

    nc.compile()
    return nc


def get_nc(B_local=2, L=2048, D=1024, H=512, reps=1, phases='all'):
    key = (B_local, L, D, H, reps, phases)
    if key not in _NC_CACHE:
        _NC_CACHE[key] = _build(*key[:4], reps=key[4], phases=key[5])
    return _NC_CACHE[key]


def kernel(a_inputs, a_mask, b_inputs, b_mask, W1, b1, W2, b2, temperature):
    from concourse.bass_utils import run_bass_kernel_spmd

    n_cores = 8
    B, L, D = a_inputs.shape
    H = W1.shape[0]
    assert B % n_cores == 0
    B_local = B // n_cores
    nc = get_nc(B_local, L, D, H)

    f32 = lambda x: np.ascontiguousarray(np.asarray(x, dtype=np.float32))
    w_shared = {
        "W1": f32(W1), "b1": f32(b1), "W2": f32(W2), "b2": f32(b2),
        "temperature": f32(temperature).reshape(1),
    }
    in_maps = []
    for c in range(n_cores):
        sl = slice(c * B_local, (c + 1) * B_local)
        in_maps.append({
            "a_inputs": f32(a_inputs[sl]),
            "a_mask": f32(a_mask[sl]),
            "b_inputs": f32(b_inputs[sl]),
            "b_mask": f32(b_mask[sl]),
            **w_shared,
        })

    res = run_bass_kernel_spmd(nc, in_maps, core_ids=list(range(n_cores)))
    full = np.concatenate([res.results[c]["out"] for c in range(n_cores)], axis=0)
    a_feature = np.ascontiguousarray(full[:, 0])
    b_feature = np.ascontiguousarray(full[:, 1])
    return (a_feature, b_feature)


# revision 26
# speedup vs baseline: 197.1258x; 1.0415x over previous
"""Trainium2 Bass kernel for the Alignment (ESIM-style) module.

Math (per batch):
    a_out = relu(a_in @ W1.T + b1); b_out = relu(b_in @ W2.T + b2)
    S = (a_out @ b_out.T) * temperature              [L, L]
    E = exp(S + ln(am_i)) * bm_j   (fully masked exp; no max-subtraction:
        scores are O(+-3) so fp32 exp is safe; exp(-1e9)==0 matches *0)
    a_feature[j, d] = sum_i E[i,j]*a_in[i,d] / sum_i E[i,j]   (bm_j cancels)
    b_feature[i, d] = sum_j E[i,j]*b_in[j,d] / sum_j E[i,j]   (am_i cancels)
    Matches the reference -1e9-mask softmax exactly for non-degenerate masks.

Sharding: pure data-parallel over batch. B=16 over 8 cores -> 2 batches/core.
All matmuls bf16 (fp32 PSUM accumulate); all transposes on the TensorEngine,
with transpose outputs packed 8-to-a-PSUM-bank and copied out in one op.
Softmax denominators come from DVE reduces (no extra matmuls).
"""

import numpy as np

_NC_CACHE = {}


def _build(B_local, L, D, H, reps=1, phases='all'):
    import concourse.mybir as mybir
    from concourse import bacc
    from concourse.tile import TileContext
    from concourse.masks import make_identity

    F32 = mybir.dt.float32
    BF16 = mybir.dt.bfloat16
    AF = mybir.ActivationFunctionType
    ALU = mybir.AluOpType

    assert L % 512 == 0 and D % 512 == 0 and H % 128 == 0
    DT = D // 128   # d-tiles (proj contraction)
    HT = H // 128   # h-tiles (scores contraction)
    LI = L // 128   # 128-row blocks of the sequence
    NC = L // 512   # 512-col chunks of the sequence
    DC = D // 512   # 512-col chunks of the feature dim
    CG = 1          # chunks per load group
    TG = min(8, DT)  # transposes packed per PSUM bank (input path)
    EG = min(8, LI)  # transposes packed per PSUM bank (E path)
    assert LI % CG == 0 and DT % TG == 0 and LI % EG == 0

    nc = bacc.Bacc()

    a_in = nc.declare_dram_parameter("a_inputs", [B_local, L, D], F32, isOutput=False)
    a_m = nc.declare_dram_parameter("a_mask", [B_local, L, 1], F32, isOutput=False)
    b_in = nc.declare_dram_parameter("b_inputs", [B_local, L, D], F32, isOutput=False)
    b_m = nc.declare_dram_parameter("b_mask", [B_local, L, 1], F32, isOutput=False)
    W1 = nc.declare_dram_parameter("W1", [H, D], F32, isOutput=False)
    b1 = nc.declare_dram_parameter("b1", [H], F32, isOutput=False)
    W2 = nc.declare_dram_parameter("W2", [H, D], F32, isOutput=False)
    b2 = nc.declare_dram_parameter("b2", [H], F32, isOutput=False)
    temp = nc.declare_dram_parameter("temperature", [1], F32, isOutput=False)
    out = nc.declare_dram_parameter("out", [B_local, 2, L, D], F32, isOutput=True)

    with TileContext(nc) as tc:
        with (
            tc.tile_pool(name="const", bufs=1) as cp,
            tc.tile_pool(name="unit", bufs=4) as up,
            tc.tile_pool(name="chunk", bufs=4) as chp,
            tc.tile_pool(name="eblk", bufs=2) as ebp,
            tc.tile_pool(name="feat", bufs=2) as fp,
            tc.tile_pool(name="bmbc", bufs=1) as bmp,
            tc.tile_pool(name="small", bufs=2) as sp,
            tc.tile_pool(name="psmm", bufs=3, space="PSUM") as psmm,
            tc.tile_pool(name="psacc", bufs=3, space="PSUM") as psacc,
            tc.tile_pool(name="pstp", bufs=2, space="PSUM") as pstp,
        ):
            # ---- constants ----
            ident = cp.tile([128, 128], BF16, tag="ident")
            make_identity(nc, ident)

            t_sb = cp.tile([1, 1], F32, tag="t_sb")
            nc.sync.dma_start(t_sb[:], temp[None, :])
            t_bc = cp.tile([128, 1], F32, tag="t_bc")
            nc.gpsimd.partition_broadcast(t_bc[:], t_sb[:])

            b1_sb = cp.tile([128, HT], F32, tag="b1_sb")
            nc.sync.dma_start(b1_sb[:], b1.rearrange("(t p) -> p t", p=128))
            b1_eff = cp.tile([128, HT], F32, tag="b1_eff")
            nc.vector.tensor_scalar_mul(b1_eff[:], b1_sb[:], t_bc[:])
            b2_sb = cp.tile([128, HT], F32, tag="b2_sb")
            nc.sync.dma_start(b2_sb[:], b2.rearrange("(t p) -> p t", p=128))

            # ---- weights: W [H, D] f32 -> WT [d%128, DT, H] bf16 (PE transpose) ----
            WTs = []
            for wi, W in enumerate((W1, W2)):
                WT = cp.tile([128, DT, H], BF16, tag=f"W{wi}T")
                for ht in range(HT):
                    wch = chp.tile([128, 1, D], F32, tag="chunk")
                    nc.sync.dma_start(wch[:, 0, :], W[ht * 128:(ht + 1) * 128, :])
                    wchb = sp.tile([128, D], BF16, tag="chunkb", bufs=3)
                    nc.vector.tensor_copy(wchb[:], wch[:, 0, :])
                    for kg in range(DT // TG):
                        ptp = pstp.tile([128, TG, 128], BF16, tag="tp")
                        for k in range(TG):
                            kt = kg * TG + k
                            nc.tensor.transpose(
                                ptp[:, k, :],
                                wchb[:, kt * 128:(kt + 1) * 128],
                                ident[:],
                            )
                        nc.vector.tensor_copy(
                            WT[:, kg * TG:(kg + 1) * TG, ht * 128:(ht + 1) * 128],
                            ptp[:],
                        )
                WTs.append(WT)
            W1T, W2T = WTs

            for rep, b in [(r, bb) for r in range(reps) for bb in range(B_local)]:
                # ---- masks ----
                am_all = sp.tile([128, LI], F32, tag="mrow")
                nc.sync.dma_start(
                    am_all[:], a_m.rearrange("b (c p) one -> b p (c one)", p=128)[b]
                )
                lnam = sp.tile([128, LI], F32, tag="lnam")
                nc.scalar.activation(lnam[:], am_all[:], AF.Ln)
                bm_row = sp.tile([1, L], F32, tag="bmrow", bufs=1)
                nc.sync.dma_start(bm_row[:], b_m.rearrange("b l one -> b (one l)")[b][None, :])
                bm_row_bf = sp.tile([1, L], BF16, tag="bmrowb", bufs=1)
                nc.vector.tensor_copy(bm_row_bf[:], bm_row[:])
                bm_bc = bmp.tile([128, L], BF16, tag="bmbc", name=f"bmbc{b}")
                nc.gpsimd.partition_broadcast(bm_bc[:], bm_row_bf[:])

                # ---- phase A: load + convert + transpose both sides ----
                nat = [None, None]
                inT = [None, None]
                for side in range(2):  # 0 = a, 1 = b
                    src = a_in if side == 0 else b_in
                    nat[side] = up.tile(
                        [128, LI, D], BF16, tag="unit", name=f"nat{side}_{b}"
                    )
                    inT[side] = up.tile(
                        [128, DT, L], BF16, tag="unit", name=f"inT{side}_{b}"
                    )
                    for g in range(LI // CG):
                        ch = chp.tile([128, CG, D], F32, tag="chunk")
                        nc.sync.dma_start(
                            ch[:],
                            src.rearrange("b (g c p) d -> b g p c d", p=128, c=CG)[b, g],
                        )
                        nc.vector.tensor_copy(
                            nat[side][:, g * CG:(g + 1) * CG, :], ch[:]
                        )
                        for c in range(g * CG, (g + 1) * CG):
                            for kg in range(DT // TG):
                                ptp = pstp.tile([128, TG, 128], BF16, tag="tp")
                                for k in range(TG):
                                    kt = kg * TG + k
                                    nc.tensor.transpose(
                                        ptp[:, k, :],
                                        nat[side][:, c, kt * 128:(kt + 1) * 128],
                                        ident[:],
                                    )
                                nc.vector.tensor_copy(
                                    inT[side][:, kg * TG:(kg + 1) * TG,
                                              c * 128:(c + 1) * 128],
                                    ptp[:],
                                )

                # ---- phase B: projections -> outT [h, i] ----
                outT = up.tile([128, 2, HT, L], BF16, tag="unit", name=f"outT{b}")
                for side in range(2):
                    WT = W1T if side == 0 else W2T
                    for ht in range(HT):
                        for icn in range(NC):
                            pm = psmm.tile([128, 512], F32, tag="mm")
                            for kt in range(DT):
                                nc.tensor.matmul(
                                    pm[:],
                                    WT[:, kt, ht * 128:(ht + 1) * 128],
                                    inT[side][:, kt, icn * 512:(icn + 1) * 512],
                                    start=(kt == 0),
                                    stop=(kt == DT - 1),
                                )
                            if side == 0:
                                # fold temperature: relu(t*x + t*b1) = t*relu(x+b1)
                                nc.scalar.activation(
                                    outT[:, 0, ht, icn * 512:(icn + 1) * 512], pm[:],
                                    AF.Relu, bias=b1_eff[:, ht:ht + 1], scale=t_bc[:],
                                )
                            else:
                                nc.scalar.activation(
                                    outT[:, 1, ht, icn * 512:(icn + 1) * 512], pm[:],
                                    AF.Relu, bias=b2_sb[:, ht:ht + 1], scale=1.0,
                                )

                if phases == 'ab':
                    # dump outT so the phase isn't dead-code eliminated
                    foq = fp.tile([128, D], F32, tag="feat")
                    nc.vector.tensor_copy(foq[:, 0:L // 2 if False else 512], outT[:, 0, 0, 0:512])
                    nc.sync.dma_start(out[b, 0, 0:128, 0:512], foq[:, 0:512])
                    continue
                # ---- pass 1 (i-outer): E row blocks (masked), b_feature ----
                E_half = [
                    up.tile([128, LI // 2, L], BF16, tag="unit", name=f"E{h}_{b}")
                    for h in range(2)
                ]
                c_acc = sp.tile([128, LI], F32, tag="cacc")
                for iblk in range(LI):
                    Et = E_half[0] if iblk < LI // 2 else E_half[1]
                    ei = iblk % (LI // 2)
                    for jc in range(NC):
                        ps = psmm.tile([128, 512], F32, tag="mm")
                        for kt in range(HT):
                            nc.tensor.matmul(
                                ps[:],
                                outT[:, 0, kt, iblk * 128:(iblk + 1) * 128],
                                outT[:, 1, kt, jc * 512:(jc + 1) * 512],
                                start=(kt == 0),
                                stop=(kt == HT - 1),
                            )
                        nc.scalar.activation(
                            Et[:, ei, jc * 512:(jc + 1) * 512], ps[:], AF.Exp,
                            bias=lnam[:, iblk:iblk + 1],
                        )
                    # row-mask by bm (in-place), then row sums r
                    # (tensor_tensor_reduce would fuse these but crashes on HW)
                    r_blk = sp.tile([128, 1], F32, tag="rblk", bufs=4)
                    nc.vector.tensor_mul(Et[:, ei, :], Et[:, ei, :], bm_bc[:])
                    nc.vector.reduce_sum(
                        r_blk[:], Et[:, ei, :], axis=mybir.AxisListType.X,
                    )
                    # transpose the (masked) row block
                    EbT = ebp.tile([128, LI, 128], BF16, tag="eblk")
                    for jg in range(LI // EG):
                        ptp = pstp.tile([128, EG, 128], BF16, tag="tp", name="ptpe")
                        for k in range(EG):
                            jt = jg * EG + k
                            nc.tensor.transpose(
                                ptp[:, k, :], Et[:, ei, jt * 128:(jt + 1) * 128],
                                ident[:],
                            )
                        nc.vector.tensor_copy(EbT[:, jg * EG:(jg + 1) * EG, :], ptp[:])
                    # column-sum contribution of this row block: c += sum_i EbT
                    c_part = sp.tile([128, LI, 1], F32, tag="cpart")
                    nc.vector.tensor_reduce(
                        c_part[:], EbT[:], axis=mybir.AxisListType.X, op=ALU.add,
                    )
                    if iblk == 0:
                        nc.vector.tensor_copy(c_acc[:], c_part[:, :, 0])
                    else:
                        nc.vector.tensor_add(c_acc[:], c_acc[:], c_part[:, :, 0])
                    # b_feature rows for this block
                    pn = [
                        psacc.tile([128, 512], F32, tag="acc", name=f"pn{dd}")
                        for dd in range(DC)
                    ]
                    for jt in range(LI):
                        for dcn in range(DC):
                            nc.tensor.matmul(
                                pn[dcn][:],
                                EbT[:, jt, :],
                                nat[1][:, jt, dcn * 512:(dcn + 1) * 512],
                                start=(jt == 0),
                                stop=(jt == LI - 1),
                            )
                    rv = sp.tile([128, 1], F32, tag="rinv", bufs=4)
                    nc.vector.reciprocal(rv[:], r_blk[:])
                    fo = fp.tile([128, D], F32, tag="feat")
                    for dcn in range(DC):
                        nc.scalar.activation(
                            fo[:, dcn * 512:(dcn + 1) * 512], pn[dcn][:],
                            AF.Copy, scale=rv[:],
                        )
                    nc.sync.dma_start(out[b, 1, iblk * 128:(iblk + 1) * 128, :], fo[:])

                if phases == 'abp1':
                    continue
                # ---- pass 2 (j-outer): a_feature (rhs = unmasked a_in, reload) ----
                nat2 = up.tile([128, LI, D], BF16, tag="unit", name=f"nat2_{b}")
                for g in range(LI // CG):
                    ch = chp.tile([128, CG, D], F32, tag="chunk")
                    nc.sync.dma_start(
                        ch[:],
                        a_in.rearrange("b (g c p) d -> b g p c d", p=128, c=CG)[b, g],
                    )
                    nc.vector.tensor_copy(nat2[:, g * CG:(g + 1) * CG, :], ch[:])
                for jblk in range(LI):
                    pp = psacc if jblk % 2 == 0 else psmm
                    tg = "acc" if jblk % 2 == 0 else "mm"
                    pn = [
                        pp.tile([128, 512], F32, tag=tg, name=f"pa{dd}")
                        for dd in range(DC)
                    ]
                    for it in range(LI):
                        Et = E_half[0] if it < LI // 2 else E_half[1]
                        ei = it % (LI // 2)
                        for dcn in range(DC):
                            nc.tensor.matmul(
                                pn[dcn][:],
                                Et[:, ei, jblk * 128:(jblk + 1) * 128],
                                nat2[:, it, dcn * 512:(dcn + 1) * 512],
                                start=(it == 0),
                                stop=(it == LI - 1),
                            )
                    cv = sp.tile([128, 1], F32, tag="rinv", bufs=4)
                    nc.vector.reciprocal(cv[:], c_acc[:, jblk:jblk + 1])
                    fo = fp.tile([128, D], F32, tag="feat")
                    for dcn in range(DC):
                        nc.scalar.activation(
                            fo[:, dcn * 512:(dcn + 1) * 512], pn[dcn][:],
                            AF.Copy, scale=cv[:],
                        )
                    nc.sync.dma_start(out[b, 0, jblk * 128:(jblk + 1) * 128, :], fo[:])

    nc.compile()
    return nc


def get_nc(B_local=2, L=2048, D=1024, H=512, reps=1, phases='all'):
    key = (B_local, L, D, H, reps, phases)
    if key not in _NC_CACHE:
        _NC_CACHE[key] = _build(*key[:4], reps=key[4], phases=key[5])
    return _NC_CACHE[key]


def kernel(a_inputs, a_mask, b_inputs, b_mask, W1, b1, W2, b2, temperature):
    from concourse.bass_utils import run_bass_kernel_spmd

    n_cores = 8
    B, L, D = a_inputs.shape
    H = W1.shape[0]
    assert B % n_cores == 0
    B_local = B // n_cores
    nc = get_nc(B_local, L, D, H)

    f32 = lambda x: np.ascontiguousarray(np.asarray(x, dtype=np.float32))
    w_shared = {
        "W1": f32(W1), "b1": f32(b1), "W2": f32(W2), "b2": f32(b2),
        "temperature": f32(temperature).reshape(1),
    }
    in_maps = []
    for c in range(n_cores):
        sl = slice(c * B_local, (c + 1) * B_local)
        in_maps.append({
            "a_inputs": f32(a_inputs[sl]),
            "a_mask": f32(a_mask[sl]),
            "b_inputs": f32(b_inputs[sl]),
            "b_mask": f32(b_mask[sl]),
            **w_shared,
        })

    res = run_bass_kernel_spmd(nc, in_maps, core_ids=list(range(n_cores)))
    full = np.concatenate([res.results[c]["out"] for c in range(n_cores)], axis=0)
    a_feature = np.ascontiguousarray(full[:, 0])
    b_feature = np.ascontiguousarray(full[:, 1])
    return (a_feature, b_feature)


# revision 32
# speedup vs baseline: 213.0796x; 1.0809x over previous
"""Trainium2 Bass kernel for the Alignment (ESIM-style) module.

Math (per batch):
    a_out = relu(a_in @ W1.T + b1); b_out = relu(b_in @ W2.T + b2)
    S = (a_out @ b_out.T) * temperature              [L, L]
    E = exp(S + ln(am_i)) * bm_j   (fully masked exp; no max-subtraction:
        scores are O(+-3) so fp32 exp is safe; exp(-1e9)==0 matches *0)
    a_feature[j, d] = sum_i E[i,j]*a_in[i,d] / sum_i E[i,j]   (bm_j cancels)
    b_feature[i, d] = sum_j E[i,j]*b_in[j,d] / sum_j E[i,j]   (am_i cancels)
    Matches the reference -1e9-mask softmax exactly for non-degenerate masks.

Sharding: pure data-parallel over batch. B=16 over 8 cores -> 2 batches/core.
All matmuls bf16 (fp32 PSUM accumulate); all transposes on the TensorEngine,
with transpose outputs packed 8-to-a-PSUM-bank and copied out in one op.
Softmax denominators come from DVE reduces (no extra matmuls).
"""

import numpy as np

_NC_CACHE = {}


def _build(B_local, L, D, H, reps=1, phases='all'):
    import concourse.mybir as mybir
    from concourse import bacc
    from concourse.tile import TileContext
    from concourse.masks import make_identity

    F32 = mybir.dt.float32
    BF16 = mybir.dt.bfloat16
    F8 = mybir.dt.float8e4
    AF = mybir.ActivationFunctionType
    ALU = mybir.AluOpType

    assert L % 512 == 0 and D % 512 == 0 and H % 128 == 0
    DT = D // 128   # d-tiles (proj contraction)
    HT = H // 128   # h-tiles (scores contraction)
    LI = L // 128   # 128-row blocks of the sequence
    NC = L // 512   # 512-col chunks of the sequence
    DC = D // 512   # 512-col chunks of the feature dim
    CG = 1          # chunks per load group
    TG = min(8, DT)  # transposes packed per PSUM bank (input path)
    EG = min(8, LI)  # transposes packed per PSUM bank (E path)
    assert LI % CG == 0 and DT % TG == 0 and LI % EG == 0

    nc = bacc.Bacc()

    a_in = nc.declare_dram_parameter("a_inputs", [B_local, L, D], F32, isOutput=False)
    a_m = nc.declare_dram_parameter("a_mask", [B_local, L, 1], F32, isOutput=False)
    b_in = nc.declare_dram_parameter("b_inputs", [B_local, L, D], F32, isOutput=False)
    b_m = nc.declare_dram_parameter("b_mask", [B_local, L, 1], F32, isOutput=False)
    W1 = nc.declare_dram_parameter("W1", [H, D], F32, isOutput=False)
    b1 = nc.declare_dram_parameter("b1", [H], F32, isOutput=False)
    W2 = nc.declare_dram_parameter("W2", [H, D], F32, isOutput=False)
    b2 = nc.declare_dram_parameter("b2", [H], F32, isOutput=False)
    temp = nc.declare_dram_parameter("temperature", [1], F32, isOutput=False)
    out = nc.declare_dram_parameter("out", [B_local, 2, L, D], F32, isOutput=True)

    with TileContext(nc) as tc:
        with (
            tc.tile_pool(name="const", bufs=1) as cp,
            tc.tile_pool(name="unit", bufs=4) as up,
            tc.tile_pool(name="chunk", bufs=4) as chp,
            tc.tile_pool(name="eblk", bufs=2) as ebp,
            tc.tile_pool(name="feat", bufs=2) as fp,
            tc.tile_pool(name="bmbc", bufs=1) as bmp,
            tc.tile_pool(name="small", bufs=2) as sp,
            tc.tile_pool(name="psmm", bufs=3, space="PSUM") as psmm,
            tc.tile_pool(name="psacc", bufs=3, space="PSUM") as psacc,
            tc.tile_pool(name="pstp", bufs=2, space="PSUM") as pstp,
        ):
            # ---- constants ----
            ident = cp.tile([128, 128], BF16, tag="ident")
            make_identity(nc, ident)

            t_sb = cp.tile([1, 1], F32, tag="t_sb")
            nc.sync.dma_start(t_sb[:], temp[None, :])
            t_bc = cp.tile([128, 1], F32, tag="t_bc")
            nc.gpsimd.partition_broadcast(t_bc[:], t_sb[:])

            b1_sb = cp.tile([128, HT], F32, tag="b1_sb")
            nc.sync.dma_start(b1_sb[:], b1.rearrange("(t p) -> p t", p=128))
            b1_eff = cp.tile([128, HT], F32, tag="b1_eff")
            nc.vector.tensor_scalar_mul(b1_eff[:], b1_sb[:], t_bc[:])
            b2_sb = cp.tile([128, HT], F32, tag="b2_sb")
            nc.sync.dma_start(b2_sb[:], b2.rearrange("(t p) -> p t", p=128))

            # ---- weights: W [H, D] f32 -> WT [d%128, DT, H] bf16 (PE transpose) ----
            WTs = []
            for wi, W in enumerate((W1, W2)):
                WT = cp.tile([128, DT, H], BF16, tag=f"W{wi}T")
                for ht in range(HT):
                    wch = chp.tile([128, 1, D], F32, tag="chunk")
                    nc.sync.dma_start(wch[:, 0, :], W[ht * 128:(ht + 1) * 128, :])
                    wchb = sp.tile([128, D], BF16, tag="chunkb", bufs=3)
                    nc.vector.tensor_copy(wchb[:], wch[:, 0, :])
                    for kg in range(DT // TG):
                        ptp = pstp.tile([128, TG, 128], BF16, tag="tp")
                        for k in range(TG):
                            kt = kg * TG + k
                            nc.tensor.transpose(
                                ptp[:, k, :],
                                wchb[:, kt * 128:(kt + 1) * 128],
                                ident[:],
                            )
                        nc.vector.tensor_copy(
                            WT[:, kg * TG:(kg + 1) * TG, ht * 128:(ht + 1) * 128],
                            ptp[:],
                        )
                WTs.append(WT)
            W1T, W2T = WTs

            for rep, b in [(r, bb) for r in range(reps) for bb in range(B_local)]:
                # ---- masks ----
                am_all = sp.tile([128, LI], F32, tag="mrow")
                nc.sync.dma_start(
                    am_all[:], a_m.rearrange("b (c p) one -> b p (c one)", p=128)[b]
                )
                lnam = sp.tile([128, LI], F32, tag="lnam")
                nc.scalar.activation(lnam[:], am_all[:], AF.Ln)
                bm_row = sp.tile([1, L], F32, tag="bmrow", bufs=1)
                nc.sync.dma_start(bm_row[:], b_m.rearrange("b l one -> b (one l)")[b][None, :])
                bm_row_bf = sp.tile([1, L], BF16, tag="bmrowb", bufs=1)
                nc.vector.tensor_copy(bm_row_bf[:], bm_row[:])
                bm_bc = bmp.tile([128, L], BF16, tag="bmbc", name=f"bmbc{b}")
                nc.gpsimd.partition_broadcast(bm_bc[:], bm_row_bf[:])

                # ---- phase A: load + convert + transpose both sides ----
                nat = [None, None]
                inT = [None, None]
                for side in range(2):  # 0 = a, 1 = b
                    src = a_in if side == 0 else b_in
                    nat[side] = up.tile(
                        [128, LI, D], BF16, tag="unit", name=f"nat{side}_{b}"
                    )
                    inT[side] = up.tile(
                        [128, DT, L], BF16, tag="unit", name=f"inT{side}_{b}"
                    )
                    for g in range(LI // CG):
                        ch = chp.tile([128, CG, D], F32, tag="chunk")
                        nc.sync.dma_start(
                            ch[:],
                            src.rearrange("b (g c p) d -> b g p c d", p=128, c=CG)[b, g],
                        )
                        nc.vector.tensor_copy(
                            nat[side][:, g * CG:(g + 1) * CG, :], ch[:]
                        )
                        for c in range(g * CG, (g + 1) * CG):
                            for kg in range(DT // TG):
                                ptp = pstp.tile([128, TG, 128], BF16, tag="tp")
                                for k in range(TG):
                                    kt = kg * TG + k
                                    nc.tensor.transpose(
                                        ptp[:, k, :],
                                        nat[side][:, c, kt * 128:(kt + 1) * 128],
                                        ident[:],
                                    )
                                nc.vector.tensor_copy(
                                    inT[side][:, kg * TG:(kg + 1) * TG,
                                              c * 128:(c + 1) * 128],
                                    ptp[:],
                                )

                # ---- phase B: projections -> outT [h, i] ----
                outT = up.tile([128, 2, HT, L], F8, tag="unit", name=f"outT{b}")
                for side in range(2):
                    WT = W1T if side == 0 else W2T
                    for ht in range(HT):
                        for icn in range(NC):
                            pm = psmm.tile([128, 512], F32, tag="mm")
                            for kt in range(DT):
                                nc.tensor.matmul(
                                    pm[:],
                                    WT[:, kt, ht * 128:(ht + 1) * 128],
                                    inT[side][:, kt, icn * 512:(icn + 1) * 512],
                                    start=(kt == 0),
                                    stop=(kt == DT - 1),
                                )
                            if side == 0:
                                nc.scalar.activation(
                                    outT[:, 0, ht, icn * 512:(icn + 1) * 512], pm[:],
                                    AF.Relu, bias=b1_sb[:, ht:ht + 1], scale=1.0,
                                )
                            else:
                                nc.scalar.activation(
                                    outT[:, 1, ht, icn * 512:(icn + 1) * 512], pm[:],
                                    AF.Relu, bias=b2_sb[:, ht:ht + 1], scale=1.0,
                                )

                if phases == 'ab':
                    # dump outT so the phase isn't dead-code eliminated
                    foq = fp.tile([128, D], F32, tag="feat")
                    nc.vector.tensor_copy(foq[:, 0:L // 2 if False else 512], outT[:, 0, 0, 0:512])
                    nc.sync.dma_start(out[b, 0, 0:128, 0:512], foq[:, 0:512])
                    continue
                # ---- pass 1 (i-outer): E row blocks (masked), b_feature ----
                E_half = [
                    up.tile([128, LI // 2, L], BF16, tag="unit", name=f"E{h}_{b}")
                    for h in range(2)
                ]
                c_acc = sp.tile([128, LI], F32, tag="cacc")
                for iblk in range(LI):
                    Et = E_half[0] if iblk < LI // 2 else E_half[1]
                    ei = iblk % (LI // 2)
                    for jc in range(NC):
                        ps = psmm.tile([128, 512], F32, tag="mm")
                        if HT % 2 == 0:
                            for k2 in range(HT // 2):
                                nc.tensor.matmul(
                                    ps[:],
                                    outT[:, 0, 2 * k2:2 * k2 + 2,
                                         iblk * 128:(iblk + 1) * 128],
                                    outT[:, 1, 2 * k2:2 * k2 + 2,
                                         jc * 512:(jc + 1) * 512],
                                    start=(k2 == 0),
                                    stop=(k2 == HT // 2 - 1),
                                    perf_mode=mybir.MatmulPerfMode.DoubleRow,
                                )
                        else:
                            for kt in range(HT):
                                nc.tensor.matmul(
                                    ps[:],
                                    outT[:, 0, kt, iblk * 128:(iblk + 1) * 128],
                                    outT[:, 1, kt, jc * 512:(jc + 1) * 512],
                                    start=(kt == 0),
                                    stop=(kt == HT - 1),
                                )
                        nc.scalar.activation(
                            Et[:, ei, jc * 512:(jc + 1) * 512], ps[:], AF.Exp,
                            bias=lnam[:, iblk:iblk + 1], scale=t_bc[:],
                        )
                    # row-mask by bm (in-place), then row sums r
                    # (tensor_tensor_reduce would fuse these but crashes on HW)
                    r_blk = sp.tile([128, 1], F32, tag="rblk", bufs=4)
                    nc.vector.tensor_mul(Et[:, ei, :], Et[:, ei, :], bm_bc[:])
                    nc.vector.reduce_sum(
                        r_blk[:], Et[:, ei, :], axis=mybir.AxisListType.X,
                    )
                    # transpose the (masked) row block
                    EbT = ebp.tile([128, LI, 128], BF16, tag="eblk")
                    for jg in range(LI // EG):
                        ptp = pstp.tile([128, EG, 128], BF16, tag="tp", name="ptpe")
                        for k in range(EG):
                            jt = jg * EG + k
                            nc.tensor.transpose(
                                ptp[:, k, :], Et[:, ei, jt * 128:(jt + 1) * 128],
                                ident[:],
                            )
                        nc.vector.tensor_copy(EbT[:, jg * EG:(jg + 1) * EG, :], ptp[:])
                    # column-sum contribution of this row block: c += sum_i EbT
                    c_part = sp.tile([128, LI, 1], F32, tag="cpart")
                    nc.vector.tensor_reduce(
                        c_part[:], EbT[:], axis=mybir.AxisListType.X, op=ALU.add,
                    )
                    if iblk == 0:
                        nc.vector.tensor_copy(c_acc[:], c_part[:, :, 0])
                    else:
                        nc.vector.tensor_add(c_acc[:], c_acc[:], c_part[:, :, 0])
                    # b_feature rows for this block
                    pn = [
                        psacc.tile([128, 512], F32, tag="acc", name=f"pn{dd}")
                        for dd in range(DC)
                    ]
                    for jt in range(LI):
                        for dcn in range(DC):
                            nc.tensor.matmul(
                                pn[dcn][:],
                                EbT[:, jt, :],
                                nat[1][:, jt, dcn * 512:(dcn + 1) * 512],
                                start=(jt == 0),
                                stop=(jt == LI - 1),
                            )
                    rv = sp.tile([128, 1], F32, tag="rinv", bufs=4)
                    nc.vector.reciprocal(rv[:], r_blk[:])
                    fo = fp.tile([128, D], F32, tag="feat")
                    for dcn in range(DC):
                        nc.scalar.activation(
                            fo[:, dcn * 512:(dcn + 1) * 512], pn[dcn][:],
                            AF.Copy, scale=rv[:],
                        )
                    nc.sync.dma_start(out[b, 1, iblk * 128:(iblk + 1) * 128, :], fo[:])

                if phases == 'abp1':
                    continue
                # ---- pass 2 (j-outer): a_feature (rhs = unmasked a_in, reload) ----
                nat2 = up.tile([128, LI, D], BF16, tag="unit", name=f"nat2_{b}")
                for g in range(LI // CG):
                    ch = chp.tile([128, CG, D], F32, tag="chunk")
                    nc.sync.dma_start(
                        ch[:],
                        a_in.rearrange("b (g c p) d -> b g p c d", p=128, c=CG)[b, g],
                    )
                    nc.vector.tensor_copy(nat2[:, g * CG:(g + 1) * CG, :], ch[:])
                for jblk in range(LI):
                    pp = psacc if jblk % 2 == 0 else psmm
                    tg = "acc" if jblk % 2 == 0 else "mm"
                    pn = [
                        pp.tile([128, 512], F32, tag=tg, name=f"pa{dd}")
                        for dd in range(DC)
                    ]
                    for it in range(LI):
                        Et = E_half[0] if it < LI // 2 else E_half[1]
                        ei = it % (LI // 2)
                        for dcn in range(DC):
                            nc.tensor.matmul(
                                pn[dcn][:],
                                Et[:, ei, jblk * 128:(jblk + 1) * 128],
                                nat2[:, it, dcn * 512:(dcn + 1) * 512],
                                start=(it == 0),
                                stop=(it == LI - 1),
                            )
                    cv = sp.tile([128, 1], F32, tag="rinv", bufs=4)
                    nc.vector.reciprocal(cv[:], c_acc[:, jblk:jblk + 1])
                    fo = fp.tile([128, D], F32, tag="feat")
                    for dcn in range(DC):
                        nc.scalar.activation(
                            fo[:, dcn * 512:(dcn + 1) * 512], pn[dcn][:],
                            AF.Copy, scale=cv[:],
                        )
                    nc.sync.dma_start(out[b, 0, jblk * 128:(jblk + 1) * 128, :], fo[:])

    nc.compile()
    return nc


def get_nc(B_local=2, L=2048, D=1024, H=512, reps=1, phases='all'):
    key = (B_local, L, D, H, reps, phases)
    if key not in _NC_CACHE:
        _NC_CACHE[key] = _build(*key[:4], reps=key[4], phases=key[5])
    return _NC_CACHE[key]


def kernel(a_inputs, a_mask, b_inputs, b_mask, W1, b1, W2, b2, temperature):
    from concourse.bass_utils import run_bass_kernel_spmd

    n_cores = 8
    B, L, D = a_inputs.shape
    H = W1.shape[0]
    assert B % n_cores == 0
    B_local = B // n_cores
    nc = get_nc(B_local, L, D, H)

    f32 = lambda x: np.ascontiguousarray(np.asarray(x, dtype=np.float32))
    w_shared = {
        "W1": f32(W1), "b1": f32(b1), "W2": f32(W2), "b2": f32(b2),
        "temperature": f32(temperature).reshape(1),
    }
    in_maps = []
    for c in range(n_cores):
        sl = slice(c * B_local, (c + 1) * B_local)
        in_maps.append({
            "a_inputs": f32(a_inputs[sl]),
            "a_mask": f32(a_mask[sl]),
            "b_inputs": f32(b_inputs[sl]),
            "b_mask": f32(b_mask[sl]),
            **w_shared,
        })

    res = run_bass_kernel_spmd(nc, in_maps, core_ids=list(range(n_cores)))
    full = np.concatenate([res.results[c]["out"] for c in range(n_cores)], axis=0)
    a_feature = np.ascontiguousarray(full[:, 0])
    b_feature = np.ascontiguousarray(full[:, 1])
    return (a_feature, b_feature)
